# revision 1
# baseline (speedup 1.0000x reference)
"""MoE layer (8 experts, top-2) on 8 Trainium2 NeuronCores — expert parallel.

Strategy
--------
Host (inside kernel(), cheap O(T*D) work):
  * gate: logits = x @ Wg, softmax, top-2, normalized combine weights
  * dispatch: gather each expert's tokens, pad to a common capacity CAP,
    pre-permute every operand into the exact on-chip layout so each DMA
    moves long contiguous runs per partition
  * combine: out[t] += w * (y + b2[e]) scatter-add

Device (one expert per core, SPMD over 8 cores, >99% of FLOPs):
  * yT = W2[e].T @ gelu(W1[e].T @ xT + b1[e])  with all matmuls on TensorE
  * weights resident in SBUF (bf16), fp32 PSUM accumulation
  * layout keeps D/F on partitions and tokens on the matmul free dim, so
    no transposes are needed anywhere on device
  * token dim split into near-equal tiles (<=512) — per-tile matmul count
    is fixed (512), so balanced tiles beat a small remainder tile which
    would run at the LDWEIGHTS floor

Returns the full [B, S, D] float32 output.
"""

import os
import sys

for _p in ("/opt/trn_rl_repo",):
    if _p not in sys.path:
        sys.path.insert(0, _p)

import numpy as np
import ml_dtypes

import concourse.bass as bass
import concourse.mybir as mybir
import concourse.tile as tile
from concourse import bacc
from concourse.bass_utils import run_bass_kernel_spmd

D_MODEL = 1024
D_FF = 4096
NUM_EXPERTS = 8
TOP_K = 2
N_CORES = 8
P = 128  # SBUF partitions

DC = D_MODEL // P   # 8 chunks of the model dim
FC = D_FF // P      # 32 chunks of the ffn dim
NQ1 = 16            # W1 load pieces (fine-grained so compute starts early)
FQ = D_FF // NQ1    # 256 ffn columns per W1 piece
FC1 = FC // NQ1     # 2 fc chunks per W1 piece
NQ2 = 8             # W2 load pieces
FCQ = FC // NQ2     # 4 fc chunks per W2 piece

LAST_EXEC_NS = None


def _install_profile_hook():
    """Provide antenv.axon_hooks (NTFF profiling) if the image lacks it."""
    import types
    import contextlib
    import ctypes
    try:
        from antenv.axon_hooks import get_axon_ntff_profile_hook  # noqa: F401
        return
    except ImportError:
        pass
    so = "/opt/axon/libaxon_pjrt.so"
    if not os.path.exists(so):
        return
    lib = ctypes.CDLL(so)
    if not hasattr(lib, "axon_start_nrt_profile"):
        return
    lib.axon_start_nrt_profile.argtypes = [ctypes.POINTER(ctypes.c_int64),
                                           ctypes.c_size_t]
    lib.axon_start_nrt_profile.restype = ctypes.c_int64
    lib.axon_stop_nrt_profile.argtypes = [ctypes.c_char_p]
    lib.axon_stop_nrt_profile.restype = ctypes.c_int64

    @contextlib.contextmanager
    def _hook(output_dir, device_ids):
        import jax
        jax.devices()
        if device_ids:
            ids = (ctypes.c_int64 * len(device_ids))(*device_ids)
            rc = lib.axon_start_nrt_profile(ids, len(device_ids))
        else:
            rc = lib.axon_start_nrt_profile(None, 0)
        try:
            yield
        finally:
            if rc == 0:
                n = lib.axon_stop_nrt_profile(str(output_dir).encode())
                print(f"profile: {n} ntff file(s) -> {output_dir}",
                      file=sys.stderr)

    mod = types.ModuleType("antenv.axon_hooks")
    mod.get_axon_ntff_profile_hook = lambda: _hook
    mod.set_axon_ntff_profile_hook = lambda h: None
    sys.modules["antenv.axon_hooks"] = mod
    import antenv
    antenv.axon_hooks = mod
    import concourse.bass_utils as _bu
    _bu.upload_artifacts = lambda tmpdir: tmpdir


def _tile_shape(max_cnt):
    """Equal even tile size (<=512) and count covering max_cnt tokens."""
    lo = max(256, max_cnt)
    n = (lo + 511) // 512
    tn = -(-lo // n)
    tn += tn % 2
    return tn, n


def _build_program(tn, ntiles):
    """SPMD program: one expert's FFN over ntiles*tn tokens, bf16 matmuls.

    Two HWDGE rings are used (each ring is FIFO): the SP ring streams the
    weights in consumption order; the ACT ring carries x/b1 in and y out.
    DRAM input layouts match SBUF exactly (long contiguous runs per
    partition):
      xT  [ntiles, P, DC, tn]:  xT[i, p, dc, t] = x[i*tn+t, dc*128+p]
      W1  [NQ1, P, DC, FQ]:  W1q[q, p, dc, f] = W1[dc*128+p, q*FQ+f]
      W2  [NQ2, P, FCQ, D]:  W2q[q, p, i, d] = W2[(q*FCQ+i)*128+p, d]
      b1  [P, FC]:           b1t[p, fc] = b1[fc*128+p]
    Output yT [D, ntiles*tn] float32 (= y.T).
    """
    cap = tn * ntiles
    bf16 = mybir.dt.bfloat16
    f32 = mybir.dt.float32
    nc = bacc.Bacc("TRN2", target_bir_lowering=False, debug=False,
                   num_devices=N_CORES)

    xT_d = nc.dram_tensor("xT", [ntiles, P, DC, tn], bf16,
                          kind="ExternalInput").ap()
    w1_d = nc.dram_tensor("W1", [NQ1, P, DC, FQ], bf16, kind="ExternalInput").ap()
    w2_d = nc.dram_tensor("W2", [NQ2, P, FCQ, D_MODEL], bf16,
                          kind="ExternalInput").ap()
    b1_d = nc.dram_tensor("b1", [P, FC], f32, kind="ExternalInput").ap()
    yT_d = nc.dram_tensor("yT", [D_MODEL, cap], f32, kind="ExternalOutput").ap()

    with tile.TileContext(nc) as tc:
        with (
            tc.tile_pool(name="wpool", bufs=1) as wpool,
            tc.tile_pool(name="hpool", bufs=1) as hpool,
            tc.tile_pool(name="ypool", bufs=4) as ypool,
            tc.tile_pool(name="ph", bufs=3, space="PSUM") as ph_pool,
            tc.tile_pool(name="py", bufs=3, space="PSUM") as py_pool,
        ):
            # ACT ring (shallow — deep queues here would block the gelus):
            # bias + token tiles only
            xst = [wpool.tile([P, DC, tn], bf16, tag=f"xs{i}", name=f"xs{i}")
                   for i in range(ntiles)]
            nc.scalar.dma_start(xst[0][:], xT_d[0])
            b1s = wpool.tile([P, FC], f32)
            nc.scalar.dma_start(b1s[:], b1_d)
            for i in range(1, ntiles):
                nc.scalar.dma_start(xst[i][:], xT_d[i])

            def x_slice(ti, dc):
                return xst[ti][:, dc, :]

            # SP ring (FIFO): W1 pieces paced to mm1's consumption; W2's
            # deadlines are loose, so one piece rides at p12 and the rest
            # follow after the last W1 piece
            w1q = [wpool.tile([P, DC, FQ], bf16, tag=f"w1q{q}",
                              name=f"w1q{q}") for q in range(NQ1)]
            w2q = [wpool.tile([P, FCQ, D_MODEL], bf16, tag=f"w2q{q}",
                              name=f"w2q{q}") for q in range(NQ2)]
            order = [(w1q[q], w1_d[q]) for q in range(13)]
            order.append((w2q[0], w2_d[0]))
            order += [(w1q[q], w1_d[q]) for q in range(13, NQ1)]
            order += [(w2q[q], w2_d[q]) for q in range(1, NQ2)]
            for wq, src in order:
                nc.sync.dma_start(wq[:], src)

            # PE warm-up: dummy matmuls on scratch data while weights load,
            # so HAM un-throttles before the first real matmul
            warm = wpool.tile([P, 256], bf16)
            nc.vector.memset(warm[:], 0.0)
            wps, _ = tc.tile([P, 256], f32, space="PSUM", name="warmps")
            for _ in range(40):
                nc.tensor.matmul(wps[:], warm[:, :P], warm[:], start=True,
                                 stop=True)

            for ti in range(ntiles):
                t0 = ti * tn
                # hT = gelu(W1.T @ x + b1), layout [F(part), tokens]
                hT = hpool.tile([P, FC, tn], bf16, tag="hT")
                for fc in range(FC):
                    ph = ph_pool.tile([P, tn], f32, tag="ph")
                    q, fi = divmod(fc, FC1)
                    for dc in range(DC):
                        nc.tensor.matmul(
                            ph[:],
                            w1q[q][:, dc, fi * P:(fi + 1) * P],
                            x_slice(ti, dc),
                            start=(dc == 0),
                            stop=(dc == DC - 1),
                        )
                    nc.scalar.activation(
                        hT[:, fc, :], ph[:],
                        mybir.ActivationFunctionType.Gelu,
                        bias=b1s[:, fc:fc + 1], scale=1.0,
                    )

                # yT = W2.T @ hT, layout [D(part), tokens]
                for dc in range(DC):
                    py = py_pool.tile([P, tn], f32, tag="py")
                    for fc in range(FC):
                        q, fi = divmod(fc, FCQ)
                        nc.tensor.matmul(
                            py[:],
                            w2q[q][:, fi, dc * P:(dc + 1) * P],
                            hT[:, fc, :],
                            start=(fc == 0),
                            stop=(fc == FC - 1),
                        )
                    yt = ypool.tile([P, tn], f32, tag="yt")
                    nc.vector.tensor_copy(yt[:], py[:])
                    nc.scalar.dma_start(yT_d[dc * P:(dc + 1) * P, t0:t0 + tn],
                                        yt[:])

    nc.compile()
    return nc


def _route(x_flat, Wg):
    """Replicate the reference gate in float64: softmax, top-2, renorm."""
    logits = x_flat.astype(np.float64) @ Wg.astype(np.float64)
    logits -= logits.max(axis=-1, keepdims=True)
    p = np.exp(logits)
    p /= p.sum(axis=-1, keepdims=True)
    order = np.argsort(-p, axis=-1, kind="stable")[:, :TOP_K]   # [T, 2]
    rows = np.arange(p.shape[0])[:, None]
    tv = p[rows, order]                                          # [T, 2]
    tvn = tv / (tv.sum(axis=-1, keepdims=True) + 1e-8)
    return order, tvn


def kernel(x, Wg, W1, b1, W2, b2):
    global LAST_EXEC_NS
    x = np.asarray(x, dtype=np.float32)
    Wg = np.asarray(Wg, dtype=np.float32)
    W1 = np.asarray(W1, dtype=np.float32)
    b1 = np.asarray(b1, dtype=np.float32)
    W2 = np.asarray(W2, dtype=np.float32)
    b2 = np.asarray(b2, dtype=np.float32)

    B, S, D = x.shape
    x_flat = x.reshape(-1, D)
    T = x_flat.shape[0]

    order, tvn = _route(x_flat, Wg)

    idx = []
    wts = []
    for e in range(NUM_EXPERTS):
        sel = np.nonzero((order == e).any(axis=1))[0]
        idx.append(sel)
        wmat = np.where(order[sel] == e, tvn[sel], 0.0)
        wts.append(wmat.sum(axis=-1))                            # [cnt]

    max_cnt = max(len(s) for s in idx)
    tn, ntiles = _tile_shape(max_cnt)
    cap = tn * ntiles

    # a Bass program object must not be re-run after lowering (re-executing
    # a reused module corrupted the device) — build fresh every call; the
    # neuron compile cache keeps repeat builds fast
    nc = _build_program(tn, ntiles)

    bf16 = ml_dtypes.bfloat16
    in_maps = []
    for e in range(NUM_EXPERTS):
        sel = idx[e]
        xT = np.zeros((P, DC, cap), dtype=bf16)
        # [cnt, D] -> [cnt, DC, P] -> [P, DC, cnt]
        xT[:, :, :len(sel)] = x_flat[sel].reshape(-1, DC, P).transpose(2, 1, 0)
        # [P, DC, cap] -> [ntiles, P, DC, tn]
        xT = np.ascontiguousarray(
            xT.reshape(P, DC, ntiles, tn).transpose(2, 0, 1, 3))
        w1e = np.ascontiguousarray(
            W1[e].reshape(DC, P, NQ1, FQ).transpose(2, 1, 0, 3)).astype(bf16)
        w2e = np.ascontiguousarray(
            W2[e].reshape(NQ2, FCQ, P, D_MODEL).transpose(0, 2, 1, 3)).astype(bf16)
        in_maps.append({
            "xT": xT,
            "W1": w1e,
            "W2": w2e,
            "b1": np.ascontiguousarray(b1[e].reshape(FC, P).T),
        })

    trace = bool(os.environ.get("MOE_TRACE"))
    _install_profile_hook()   # also covers a harness-set BASS_TRACE=1
    try:
        res = run_bass_kernel_spmd(
            nc, in_maps, list(range(N_CORES)),
            trace=trace,
            tmpdir=os.environ.get("MOE_TRACE_DIR") or None,
        )
    except Exception:
        if not (trace or os.environ.get("BASS_TRACE")):
            raise
        # profiling path failed (e.g. no NTFF support) — run without it
        os.environ["BASS_NEVER_TRACE"] = "1"
        res = run_bass_kernel_spmd(nc, in_maps, list(range(N_CORES)))
    LAST_EXEC_NS = res.exec_time_ns

    out = np.zeros((T, D_MODEL), dtype=np.float64)
    for e in range(NUM_EXPERTS):
        sel = idx[e]
        yT = np.asarray(res.results[e]["yT"])                    # [D, cap] f32
        y = yT[:, :len(sel)].T.astype(np.float64)
        out[sel] += wts[e][:, None] * (y + b2[e].astype(np.float64))

    return out.reshape(B, S, D_MODEL).astype(np.float32)

